# revision 17
# baseline (speedup 1.0000x reference)
"""Multi-head attention (B=2, S=2048, D=1024, H=16) on one TRN2 chip (8 cores).

Sharding (Megatron-style): DP=2 over batch x TP=4 over heads.
Core c (c = 0..7): batch g = c//4, heads [4r, 4r+4) where r = c%4.

Per-core pipeline (inputs are host-transposed to x^T [D, S] so no
on-device transposition is needed; all matmuls run in fp32r --
single-pass fp32, full PE rate, ~19-bit mantissa):
  - Q^T/K^T [256, S] and V [S, 256] projections (fp32 accum in PSUM).
  - attention per head in "scores transposed" layout (scores^T[k, q]):
    softmax without max-subtraction (logits are O(1) here), with the
    denominator obtained for free by augmenting V with a ones column.
  - partial output projection chunk-by-chunk, each chunk
    immediately ReduceScattered(add) over the 4-core DP group so the
    collective overlaps the next chunk's compute.
Host assembles the 8 cores' shard chunks and adds the output bias.

Mask handling (kernel inspects the mask input on the host):
  - canonical causal mask -> fast path: upper-triangle key blocks
    skipped, diagonal blocks get an on-device generated additive mask.
  - all-zeros mask -> dense path, no mask applied.
  - anything else -> generic path: mask^T * sqrt(DH) streamed from DRAM
    and added to every score tile (matches exp(s*scale + m) exactly).
"""

from contextlib import ExitStack

import numpy as np

import concourse.bacc as bacc
import concourse.mybir as mybir
import concourse.tile as tile
from concourse.bass_utils import run_bass_kernel_spmd

F32 = mybir.dt.float32
F32R = mybir.dt.float32r
BF16 = mybir.dt.bfloat16
AF = mybir.ActivationFunctionType

H = 16
D = 1024
B = 2
S = 2048
DH = 64
N_CORES = 8
DP = 2                      # data-parallel groups (over batch)
TP = N_CORES // DP          # tensor-parallel cores per group
HPC = H // TP               # heads per core = 4
DHH = HPC * DH              # 256 features per core
NEG = -1e9

P = 128                     # partitions
FD = 512                    # matmul moving free dim (one PSUM bank fp32)


def _emit(tc, io, mask_mode, s, mm_dtype):
    with ExitStack() as _stk:
        _emit_inner(_stk, tc, io, mask_mode, s, mm_dtype)


def _emit_inner(stk, tc, io, mask_mode, s, mm_dtype):
    nc = tc.nc
    NQ = s // FD            # query chunks
    NK = s // P             # key tiles
    NS = s // P             # seq tiles
    ND = D // P             # d-model tiles = 8
    NH2 = HPC // 2          # head pairs = 2

    MDT = {"f32r": F32R, "bf16": BF16, "f32": F32}[mm_dtype]
    CDT = F32 if MDT != BF16 else BF16   # collective / partial dtype

    const = stk.enter_context(tc.tile_pool(name="const", bufs=1))
    persist = stk.enter_context(tc.tile_pool(name="persist", bufs=1))
    dram = stk.enter_context(tc.tile_pool(name="dram", bufs=1, space="DRAM"))

    # ---- constants -------------------------------------------------------
    ones_f32 = const.tile([1, FD], F32)
    nc.vector.memset(ones_f32, 1.0)
    ones = const.tile([1, FD], MDT)
    nc.vector.tensor_copy(ones, ones_f32)
    onescol = const.tile([P, 1], F32)
    nc.vector.memset(onescol, 1.0)

    if mask_mode == "causal":
        # diag mask tile j: allowed (0) iff  qf - kp - 128*j >= 0 else NEG
        dmask = const.tile([P, 4, FD], F32)
        nc.gpsimd.memset(dmask, 0.0)
        for j in range(4):
            nc.gpsimd.affine_select(
                out=dmask[:, j, :],
                in_=dmask[:, j, :],
                compare_op=mybir.AluOpType.is_ge,
                fill=NEG,
                base=-P * j,
                pattern=[[1, FD]],
                channel_multiplier=-1,
            )

    # ---- weights / biases (bf16 for projections, MDT for Wo) ------------
    def load_w(dst, ap):
        if MDT == BF16:
            nc.gpsimd.dma_start(dst, ap)          # SWDGE casts f32 -> bf16
        else:
            nc.sync.dma_start(dst, ap.bitcast(MDT))

    w_sb = {}
    for name in ("wq", "wk", "wv"):
        w_sb[name] = persist.tile([P, ND, DHH], MDT, name=f"w_{name}")
        load_w(w_sb[name], io[name].rearrange("(a p) o -> p a o", p=P))
    wo_sb = persist.tile([P, DHH // P, D], MDT)
    load_w(wo_sb, io["wo"].rearrange("(a p) o -> p a o", p=P))

    b_sb = {}
    for name in ("bq", "bk", "bv"):
        b_sb[name] = const.tile([1, DHH], MDT, name=f"b_{name}")
        load_w(b_sb[name], io[name])

    # ---- persistent activations: one tile per seq-chunk so attention
    # can start as soon as the chunks it reads are projected ------------
    NQl = s // FD
    SPC = FD // P                            # seq-tiles per chunk = 4
    qT = [persist.tile([P, NH2, FD], MDT, name=f"qT{i}") for i in range(NQl)]
    kT = [persist.tile([P, NH2, FD], MDT, name=f"kT{i}") for i in range(NQl)]
    v_c = [persist.tile([P, SPC, HPC, DH + 1], MDT, name=f"v{i}")
           for i in range(NQl)]
    for i in range(NQl):                     # fill the ones columns
        nc.vector.tensor_copy(
            v_c[i][:, :, :, DH:DH + 1], onescol.to_broadcast((P, SPC, HPC, 1))
        )
    ctxT = [persist.tile([P, NH2, FD], MDT, name=f"ctxT{i}")
            for i in range(NQl)]

    # ======================================================================
    # Phase A: load x^T chunks (host-pretransposed) + Q/K/V projections
    # ======================================================================
    with (
        tc.tile_pool(name="xt", bufs=2) as xt_pool,
        tc.tile_pool(name="proj_ps", bufs=2, space="PSUM") as proj_ps_pool,
        tc.tile_pool(name="vps", bufs=2, space="PSUM") as vps_pool,
    ):
        for sc in range(NQ):
            for tname, wname, bname, dstT in (
                ("xq", "wq", "bq", qT),
                ("xk", "wk", "bk", kT),
                ("xv", "wv", "bv", None),
            ):
                # x^T chunk [D, FD] as ND partition-tiles, one strided DMA
                xt_c = xt_pool.tile([P, ND, FD], MDT, tag="xt")
                xsrc = io[tname].rearrange("(a p) t -> p a t", p=P)[
                    :, :, sc * FD:(sc + 1) * FD
                ]
                if MDT == BF16:
                    nc.gpsimd.dma_start(xt_c, xsrc)
                else:
                    nc.sync.dma_start(xt_c, xsrc.bitcast(MDT))

                if dstT is not None:
                    # Q^T/K^T: out = W^T x^T  -> [features, seq-chunk]
                    for mt in range(NH2):
                        qps = proj_ps_pool.tile([P, FD], F32, tag="qps")
                        for dt in range(ND):
                            nc.tensor.matmul(
                                qps,
                                w_sb[wname][:, dt, mt * P:(mt + 1) * P],
                                xt_c[:, dt, :],
                                start=(dt == 0),
                                stop=False,
                            )
                        nc.tensor.matmul(  # + bias (ones-row augmentation)
                            qps,
                            b_sb[bname][0:1, mt * P:(mt + 1) * P],
                            ones[0:1, :],
                            start=False,
                            stop=True,
                        )
                        nc.any.tensor_copy(dstT[sc][:, mt, :], qps)
                else:
                    # V: out = x W  -> [seq, features] (native)
                    for st in range(SPC):
                        vp = vps_pool.tile([P, DHH], F32, tag="vps")
                        for dt in range(ND):
                            nc.tensor.matmul(
                                vp,
                                xt_c[:, dt, st * P:(st + 1) * P],
                                w_sb[wname][:, dt, :],
                                start=(dt == 0),
                                stop=False,
                            )
                        nc.tensor.matmul(
                            vp,
                            ones[0:1, 0:P],
                            b_sb[bname][0:1, :],
                            start=False,
                            stop=True,
                        )
                        nc.vector.tensor_copy(
                            v_c[sc][:, st, :, 0:DH],
                            vp.rearrange("p (h e) -> p h e", h=HPC),
                        )

    # ======================================================================
    # Phase B+C+D interleaved per query chunk: attention -> out-proj -> RS
    # ======================================================================
    scale = 1.0 / float(np.sqrt(DH))
    partial = dram.tile([s, D], CDT, name="partial")
    groups = [list(range(g * TP, (g + 1) * TP)) for g in range(DP)]

    with (
        tc.tile_pool(name="qk_ps", bufs=4, space="PSUM") as qk_ps_pool,
        tc.tile_pool(name="ctx_ps", bufs=4, space="PSUM") as ctx_ps_pool,
        tc.tile_pool(name="pt", bufs=8) as pt_pool,
        tc.tile_pool(name="mload", bufs=3) as mload_pool,
        tc.tile_pool(name="small", bufs=4) as small_pool,
        tc.tile_pool(name="bc_sb", bufs=4) as bc_sb_pool,
        tc.tile_pool(name="out_sb", bufs=3) as out_sb_pool,
    ):
        for qc in range(NQ):
            nkt = (qc + 1) * (FD // P) if mask_mode == "causal" else NK
            ctx = [
                ctx_ps_pool.tile([DH + 1, FD], F32, tag="ctx",
                                 name=f"ctx_{qc}_{hj}")
                for hj in range(4)
            ]
            for kt in range(nkt):
                dj = kt - qc * (FD // P)
                mt_sb = None
                if mask_mode == "generic":
                    mt_sb = mload_pool.tile([P, FD], F32, tag="ml")
                    nc.sync.dma_start(
                        mt_sb,
                        io["maskT"][kt * P:(kt + 1) * P,
                                    qc * FD:(qc + 1) * FD],
                    )
                for hj in range(4):
                    hp, j = hj // 2, hj % 2
                    ksc, kti = kt // (FD // P), kt % (FD // P)
                    sp = qk_ps_pool.tile([P, FD], F32, tag="sc",
                                         name=f"sc_{qc}_{kt}_{hj}")
                    nc.tensor.matmul(
                        sp,
                        kT[ksc][64 * j:64 * (j + 1), hp,
                                kti * P:(kti + 1) * P],
                        qT[qc][64 * j:64 * (j + 1), hp, :],
                        start=True,
                        stop=True,
                    )
                    if mt_sb is not None:
                        nc.vector.tensor_add(sp, sp, mt_sb)
                    elif mask_mode == "causal" and dj >= 0:
                        nc.vector.tensor_add(sp, sp, dmask[:, dj, :])
                    pt = pt_pool.tile([P, FD], MDT, tag="pt")
                    nc.scalar.activation(pt, sp, AF.Exp, scale=scale)
                    nc.tensor.matmul(
                        ctx[hj],
                        v_c[ksc][:, kti, hj, :],
                        pt,
                        start=(kt == 0),
                        stop=(kt == nkt - 1),
                    )
            # normalize: rows 0..63 raw ctx^T, row 64 softmax denominator.
            # (partition_broadcast only honors a base-0 input partition on
            # hardware, so each denominator gets its own [1, FD] recip.)
            for hj in range(4):
                hp, j = hj // 2, hj % 2
                den = small_pool.tile([1, FD], F32, tag="den")
                nc.scalar.copy(den, ctx[hj][DH:DH + 1, :])
                recip = small_pool.tile([1, FD], F32, tag="recip")
                nc.vector.reciprocal(recip, den)
                bc = bc_sb_pool.tile([DH, FD], F32, tag="bc")
                nc.gpsimd.partition_broadcast(bc, recip)
                nc.vector.tensor_mul(
                    ctxT[qc][64 * j:64 * (j + 1), hp, :],
                    ctx[hj][0:DH, :],
                    bc,
                )
            # out-projection for this chunk's 4 seq-tiles, then its RS
            for st in range(FD // P):
                ss = qc * (FD // P) + st
                for oc in range(D // FD):
                    op = qk_ps_pool.tile([P, FD], F32, tag="sc",
                                         name=f"op_{qc}_{st}_{oc}")
                    for hp in range(NH2):
                        nc.tensor.matmul(
                            op,
                            ctxT[qc][:, hp, st * P:(st + 1) * P],
                            wo_sb[:, hp, oc * FD:(oc + 1) * FD],
                            start=(hp == 0),
                            stop=(hp == NH2 - 1),
                        )
                    ob = out_sb_pool.tile([P, FD], CDT, tag="ob")
                    nc.any.tensor_copy(ob, op)
                    nc.sync.dma_start(
                        partial[ss * P:(ss + 1) * P, oc * FD:(oc + 1) * FD],
                        ob,
                    )
            shard_c = dram.tile([FD // TP, D], CDT, name=f"shard_{qc}")
            nc.gpsimd.collective_compute(
                "ReduceScatter",
                mybir.AluOpType.add,
                replica_groups=groups,
                ins=[partial[qc * FD:(qc + 1) * FD, :].opt()],
                outs=[shard_c.opt()],
            )
            if CDT == BF16:
                nc.gpsimd.dma_start(io["out"][qc], shard_c)  # casts to f32
            else:
                nc.sync.dma_start(io["out"][qc], shard_c)


def build(mask_mode="causal", s=S, mm_dtype="f32r"):
    """Build the SPMD Bass module for one core."""
    assert mask_mode in ("causal", "zeros", "generic")
    assert mm_dtype in ("f32r", "bf16", "f32")
    assert s % FD == 0
    nc = bacc.Bacc(
        "TRN2", target_bir_lowering=False, debug=False, num_devices=N_CORES
    )
    io = {}
    for name in ("xq", "xk", "xv"):
        # host passes x^T: [D, s]
        io[name] = nc.dram_tensor(name, [D, s], F32, kind="ExternalInput").ap()
    for name in ("wq", "wk", "wv"):
        io[name] = nc.dram_tensor(name, [D, DHH], F32, kind="ExternalInput").ap()
    io["wo"] = nc.dram_tensor("wo", [DHH, D], F32, kind="ExternalInput").ap()
    for name in ("bq", "bk", "bv"):
        io[name] = nc.dram_tensor(name, [1, DHH], F32, kind="ExternalInput").ap()
    if mask_mode == "generic":
        io["maskT"] = nc.dram_tensor(
            "maskT", [s, s], F32, kind="ExternalInput"
        ).ap()
    # output: per query-chunk shard pieces [NQ, FD/TP=128, D]
    io["out"] = nc.dram_tensor(
        "out", [s // FD, FD // TP, D], F32, kind="ExternalOutput"
    ).ap()

    with tile.TileContext(nc) as tc:
        _emit(tc, io, mask_mode, s, mm_dtype)
    nc.compile()
    return nc


def detect_mask_mode(mask, s=S):
    m = np.asarray(mask).reshape(s, s)
    if not np.any(m):
        return "zeros"
    causal = np.where(
        np.tril(np.ones((s, s), dtype=bool)), 0.0, np.float32(NEG)
    ).astype(np.float32)
    if np.array_equal(m, causal):
        return "causal"
    return "generic"


def make_in_maps(q, k, v, mask, Wq, bq, Wk, bk, Wv, bv, Wo, bo, mask_mode,
                 s=S):
    c32 = lambda a: np.ascontiguousarray(a, dtype=np.float32)
    # one host-side transpose per (batch, tensor), shared by the TP group
    xT = [[c32(np.asarray(t)[g].T) for t in (q, k, v)] for g in range(DP)]
    in_maps = []
    for c in range(N_CORES):
        g, r = c // TP, c % TP
        sl = slice(r * DHH, (r + 1) * DHH)
        m = {
            "xq": xT[g][0], "xk": xT[g][1], "xv": xT[g][2],
            "wq": c32(Wq[:, sl]), "wk": c32(Wk[:, sl]), "wv": c32(Wv[:, sl]),
            "wo": c32(Wo[sl, :]),
            "bq": c32(bq[sl]).reshape(1, DHH),
            "bk": c32(bk[sl]).reshape(1, DHH),
            "bv": c32(bv[sl]).reshape(1, DHH),
        }
        if mask_mode == "generic":
            # pre-scaled by sqrt(DH) so exp((s + m*8)/8) == exp(s/8 + m)
            m["maskT"] = c32(
                np.asarray(mask).reshape(s, s).T * np.float32(DH) ** 0.5
            )
        in_maps.append(m)
    return in_maps


def assemble(results, bo, s=S):
    out = np.empty((B, s, D), np.float32)
    piece = FD // TP  # 128 rows per (chunk, core)
    for c in range(N_CORES):
        g, r = c // TP, c % TP
        shard = np.asarray(results[c]["out"]).reshape(-1, piece, D)
        for qc in range(s // FD):
            out[g, qc * FD + r * piece:qc * FD + (r + 1) * piece, :] = (
                shard[qc]
            )
    out += np.asarray(bo, dtype=np.float32)[None, None, :]
    return out


_cache = {}
MM_DTYPE = "f32r"   # default compute dtype for kernel()


def kernel(q, k, v, mask, Wq, bq, Wk, bk, Wv, bv, Wo, bo):
    mask_mode = detect_mask_mode(mask)
    if mask_mode not in _cache:
        _cache[mask_mode] = build(mask_mode=mask_mode, mm_dtype=MM_DTYPE)
    nc = _cache[mask_mode]
    in_maps = make_in_maps(
        q, k, v, mask, Wq, bq, Wk, bk, Wv, bv, Wo, bo, mask_mode
    )
    res = run_bass_kernel_spmd(nc, in_maps, list(range(N_CORES)))
    return assemble(res.results, bo)


# revision 18
# speedup vs baseline: 1.0346x; 1.0346x over previous
"""Multi-head attention (B=2, S=2048, D=1024, H=16) on one TRN2 chip (8 cores).

Sharding (Megatron-style): DP=2 over batch x TP=4 over heads.
Core c (c = 0..7): batch g = c//4, heads [4r, 4r+4) where r = c%4.

Per-core pipeline (inputs are host-transposed to x^T [D, S] so no
on-device transposition is needed; all matmuls run in fp32r --
single-pass fp32, full PE rate, ~19-bit mantissa):
  - Q^T/K^T [256, S] and V [S, 256] projections (fp32 accum in PSUM).
  - attention per head in "scores transposed" layout (scores^T[k, q]):
    softmax without max-subtraction (logits are O(1) here), with the
    denominator obtained for free by augmenting V with a ones column.
  - partial output projection chunk-by-chunk, each chunk
    immediately ReduceScattered(add) over the 4-core DP group so the
    collective overlaps the next chunk's compute.
Host assembles the 8 cores' shard chunks and adds the output bias.

Mask handling (kernel inspects the mask input on the host):
  - canonical causal mask -> fast path: upper-triangle key blocks
    skipped, diagonal blocks get an on-device generated additive mask.
  - all-zeros mask -> dense path, no mask applied.
  - anything else -> generic path: mask^T * sqrt(DH) streamed from DRAM
    and added to every score tile (matches exp(s*scale + m) exactly).
"""

from contextlib import ExitStack

import numpy as np

import concourse.bacc as bacc
import concourse.mybir as mybir
import concourse.tile as tile
from concourse.bass_utils import run_bass_kernel_spmd

F32 = mybir.dt.float32
F32R = mybir.dt.float32r
BF16 = mybir.dt.bfloat16
AF = mybir.ActivationFunctionType

H = 16
D = 1024
B = 2
S = 2048
DH = 64
N_CORES = 8
DP = 2                      # data-parallel groups (over batch)
TP = N_CORES // DP          # tensor-parallel cores per group
HPC = H // TP               # heads per core = 4
DHH = HPC * DH              # 256 features per core
NEG = -1e9

P = 128                     # partitions
FD = 512                    # matmul moving free dim (one PSUM bank fp32)


def _emit(tc, io, mask_mode, s, mm_dtype):
    with ExitStack() as _stk:
        _emit_inner(_stk, tc, io, mask_mode, s, mm_dtype)


def _emit_inner(stk, tc, io, mask_mode, s, mm_dtype):
    nc = tc.nc
    NQ = s // FD            # query chunks
    NK = s // P             # key tiles
    ND = D // P             # d-model tiles = 8
    NH2 = HPC // 2          # head pairs = 2
    SPC = FD // P           # seq-tiles per chunk = 4

    MDT = {"f32r": F32R, "bf16": BF16, "f32": F32}[mm_dtype]
    CDT = F32 if MDT != BF16 else BF16   # collective / partial dtype

    const = stk.enter_context(tc.tile_pool(name="const", bufs=1))
    persist = stk.enter_context(tc.tile_pool(name="persist", bufs=1))
    dram = stk.enter_context(tc.tile_pool(name="dram", bufs=1, space="DRAM"))

    # ---- constants -------------------------------------------------------
    ones_f32 = const.tile([1, FD], F32)
    nc.vector.memset(ones_f32, 1.0)
    ones = const.tile([1, FD], MDT)
    nc.vector.tensor_copy(ones, ones_f32)
    onescol = const.tile([P, 1], F32)
    nc.vector.memset(onescol, 1.0)

    if mask_mode == "causal":
        # diag mask tile j: allowed (0) iff  qf - kp - 128*j >= 0 else NEG
        dmask = const.tile([P, 4, FD], F32)
        nc.gpsimd.memset(dmask, 0.0)
        for j in range(4):
            nc.gpsimd.affine_select(
                out=dmask[:, j, :],
                in_=dmask[:, j, :],
                compare_op=mybir.AluOpType.is_ge,
                fill=NEG,
                base=-P * j,
                pattern=[[1, FD]],
                channel_multiplier=-1,
            )

    # ---- weights / biases -----------------------------------------------
    def load_w(dst, ap):
        if MDT == BF16:
            nc.gpsimd.dma_start(dst, ap)          # SWDGE casts f32 -> bf16
        else:
            nc.sync.dma_start(dst, ap.bitcast(MDT))

    w_sb = {}
    for name in ("wq", "wk", "wv"):
        w_sb[name] = persist.tile([P, ND, DHH], MDT, name=f"w_{name}")
        load_w(w_sb[name], io[name].rearrange("(a p) o -> p a o", p=P))
    wo_sb = persist.tile([P, DHH // P, D], MDT)
    load_w(wo_sb, io["wo"].rearrange("(a p) o -> p a o", p=P))

    b_sb = {}
    for name in ("bq", "bk", "bv"):
        b_sb[name] = const.tile([1, DHH], MDT, name=f"b_{name}")
        load_w(b_sb[name], io[name])

    # ---- persistent activations: one tile per seq-chunk -----------------
    qT = [persist.tile([P, NH2, FD], MDT, name=f"qT{i}") for i in range(NQ)]
    kT = [persist.tile([P, NH2, FD], MDT, name=f"kT{i}") for i in range(NQ)]
    v_c = [persist.tile([P, SPC, HPC, DH + 1], MDT, name=f"v{i}")
           for i in range(NQ)]
    for i in range(NQ):                     # fill the ones columns
        nc.vector.tensor_copy(
            v_c[i][:, :, :, DH:DH + 1], onescol.to_broadcast((P, SPC, HPC, 1))
        )
    ctxT = [persist.tile([P, NH2, FD], MDT, name=f"ctxT{i}")
            for i in range(NQ)]

    scale = 1.0 / float(np.sqrt(DH))
    partial = dram.tile([s, D], CDT, name="partial")
    groups = [list(range(g * TP, (g + 1) * TP)) for g in range(DP)]

    with (
        tc.tile_pool(name="xt", bufs=2) as xt_pool,
        tc.tile_pool(name="mm_ps", bufs=4, space="PSUM") as mm_ps_pool,
        tc.tile_pool(name="ctx_ps", bufs=4, space="PSUM") as ctx_ps_pool,
        tc.tile_pool(name="pt", bufs=8) as pt_pool,
        tc.tile_pool(name="mload", bufs=3) as mload_pool,
        tc.tile_pool(name="small", bufs=4) as small_pool,
        tc.tile_pool(name="bc_sb", bufs=4) as bc_sb_pool,
        tc.tile_pool(name="out_sb", bufs=3) as out_sb_pool,
    ):
        def project_chunk(sc):
            for tname, wname, bname, dstT in (
                ("xq", "wq", "bq", qT),
                ("xk", "wk", "bk", kT),
                ("xv", "wv", "bv", None),
            ):
                xt_c = xt_pool.tile([P, ND, FD], MDT, tag="xt",
                                    name=f"xt_{tname}_{sc}")
                xsrc = io[tname].rearrange("(a p) t -> p a t", p=P)[
                    :, :, sc * FD:(sc + 1) * FD
                ]
                if MDT == BF16:
                    nc.gpsimd.dma_start(xt_c, xsrc)
                else:
                    nc.sync.dma_start(xt_c, xsrc.bitcast(MDT))

                if dstT is not None:
                    for mt in range(NH2):
                        qps = mm_ps_pool.tile([P, FD], F32, tag="mm",
                                              name=f"qps_{tname}_{sc}_{mt}")
                        for dt in range(ND):
                            nc.tensor.matmul(
                                qps,
                                w_sb[wname][:, dt, mt * P:(mt + 1) * P],
                                xt_c[:, dt, :],
                                start=(dt == 0),
                                stop=False,
                            )
                        nc.tensor.matmul(  # + bias (ones-row augmentation)
                            qps,
                            b_sb[bname][0:1, mt * P:(mt + 1) * P],
                            ones[0:1, :],
                            start=False,
                            stop=True,
                        )
                        nc.any.tensor_copy(dstT[sc][:, mt, :], qps)
                else:
                    for st in range(SPC):
                        vp = mm_ps_pool.tile([P, DHH], F32, tag="mm",
                                             name=f"vps_{sc}_{st}")
                        for dt in range(ND):
                            nc.tensor.matmul(
                                vp,
                                xt_c[:, dt, st * P:(st + 1) * P],
                                w_sb[wname][:, dt, :],
                                start=(dt == 0),
                                stop=False,
                            )
                        nc.tensor.matmul(
                            vp,
                            ones[0:1, 0:P],
                            b_sb[bname][0:1, :],
                            start=False,
                            stop=True,
                        )
                        nc.vector.tensor_copy(
                            v_c[sc][:, st, :, 0:DH],
                            vp.rearrange("p (h e) -> p h e", h=HPC),
                        )

        def attend_chunk(qc):
            nkt = (qc + 1) * SPC if mask_mode == "causal" else NK
            ctx = [
                ctx_ps_pool.tile([DH + 1, FD], F32, tag="ctx",
                                 name=f"ctx_{qc}_{hj}")
                for hj in range(4)
            ]
            for kt in range(nkt):
                ksc, kti = kt // SPC, kt % SPC
                dj = kt - qc * SPC
                mt_sb = None
                if mask_mode == "generic":
                    mt_sb = mload_pool.tile([P, FD], F32, tag="ml")
                    nc.sync.dma_start(
                        mt_sb,
                        io["maskT"][kt * P:(kt + 1) * P,
                                    qc * FD:(qc + 1) * FD],
                    )
                for hj in range(4):
                    hp, j = hj // 2, hj % 2
                    sp = mm_ps_pool.tile([P, FD], F32, tag="mm",
                                         name=f"sc_{qc}_{kt}_{hj}")
                    nc.tensor.matmul(
                        sp,
                        kT[ksc][64 * j:64 * (j + 1), hp,
                                kti * P:(kti + 1) * P],
                        qT[qc][64 * j:64 * (j + 1), hp, :],
                        start=True,
                        stop=True,
                    )
                    if mt_sb is not None:
                        nc.vector.tensor_add(sp, sp, mt_sb)
                    elif mask_mode == "causal" and dj >= 0:
                        nc.vector.tensor_add(sp, sp, dmask[:, dj, :])
                    pt = pt_pool.tile([P, FD], MDT, tag="pt")
                    nc.scalar.activation(pt, sp, AF.Exp, scale=scale)
                    nc.tensor.matmul(
                        ctx[hj],
                        v_c[ksc][:, kti, hj, :],
                        pt,
                        start=(kt == 0),
                        stop=(kt == nkt - 1),
                    )
            # normalize: rows 0..63 raw ctx^T, row 64 softmax denominator
            for hj in range(4):
                hp, j = hj // 2, hj % 2
                den = small_pool.tile([1, FD], F32, tag="den")
                nc.scalar.copy(den, ctx[hj][DH:DH + 1, :])
                recip = small_pool.tile([1, FD], F32, tag="recip")
                nc.vector.reciprocal(recip, den)
                bc = bc_sb_pool.tile([DH, FD], F32, tag="bc")
                nc.gpsimd.partition_broadcast(bc, recip)
                nc.vector.tensor_mul(
                    ctxT[qc][64 * j:64 * (j + 1), hp, :],
                    ctx[hj][0:DH, :],
                    bc,
                )

        def project_out_chunk(qc):
            for st in range(SPC):
                ss = qc * SPC + st
                for oc in range(D // FD):
                    op = mm_ps_pool.tile([P, FD], F32, tag="mm",
                                         name=f"op_{qc}_{st}_{oc}")
                    for hp in range(NH2):
                        nc.tensor.matmul(
                            op,
                            ctxT[qc][:, hp, st * P:(st + 1) * P],
                            wo_sb[:, hp, oc * FD:(oc + 1) * FD],
                            start=(hp == 0),
                            stop=(hp == NH2 - 1),
                        )
                    ob = out_sb_pool.tile([P, FD], CDT, tag="ob")
                    nc.any.tensor_copy(ob, op)
                    nc.sync.dma_start(
                        partial[ss * P:(ss + 1) * P, oc * FD:(oc + 1) * FD],
                        ob,
                    )
            shard_c = dram.tile([FD // TP, D], CDT, name=f"shard_{qc}")
            nc.gpsimd.collective_compute(
                "ReduceScatter",
                mybir.AluOpType.add,
                replica_groups=groups,
                ins=[partial[qc * FD:(qc + 1) * FD, :].opt()],
                outs=[shard_c.opt()],
            )
            if CDT == BF16:
                nc.gpsimd.dma_start(io["out"][qc], shard_c)  # casts to f32
            else:
                nc.sync.dma_start(io["out"][qc], shard_c)

        if mask_mode == "causal":
            # stream: chunk qc's attention needs only K/V chunks <= qc, so
            # interleave projection and attention per chunk -- keeps every
            # engine fed from ~the start.
            for sc in range(NQ):
                project_chunk(sc)
                attend_chunk(sc)
                project_out_chunk(sc)
        else:
            for sc in range(NQ):
                project_chunk(sc)
            for qc in range(NQ):
                attend_chunk(qc)
                project_out_chunk(qc)


def build(mask_mode="causal", s=S, mm_dtype="f32r"):
    """Build the SPMD Bass module for one core."""
    assert mask_mode in ("causal", "zeros", "generic")
    assert mm_dtype in ("f32r", "bf16", "f32")
    assert s % FD == 0
    nc = bacc.Bacc(
        "TRN2", target_bir_lowering=False, debug=False, num_devices=N_CORES
    )
    io = {}
    for name in ("xq", "xk", "xv"):
        # host passes x^T: [D, s]
        io[name] = nc.dram_tensor(name, [D, s], F32, kind="ExternalInput").ap()
    for name in ("wq", "wk", "wv"):
        io[name] = nc.dram_tensor(name, [D, DHH], F32, kind="ExternalInput").ap()
    io["wo"] = nc.dram_tensor("wo", [DHH, D], F32, kind="ExternalInput").ap()
    for name in ("bq", "bk", "bv"):
        io[name] = nc.dram_tensor(name, [1, DHH], F32, kind="ExternalInput").ap()
    if mask_mode == "generic":
        io["maskT"] = nc.dram_tensor(
            "maskT", [s, s], F32, kind="ExternalInput"
        ).ap()
    # output: per query-chunk shard pieces [NQ, FD/TP=128, D]
    io["out"] = nc.dram_tensor(
        "out", [s // FD, FD // TP, D], F32, kind="ExternalOutput"
    ).ap()

    with tile.TileContext(nc) as tc:
        _emit(tc, io, mask_mode, s, mm_dtype)
    nc.compile()
    return nc


def detect_mask_mode(mask, s=S):
    m = np.asarray(mask).reshape(s, s)
    if not np.any(m):
        return "zeros"
    causal = np.where(
        np.tril(np.ones((s, s), dtype=bool)), 0.0, np.float32(NEG)
    ).astype(np.float32)
    if np.array_equal(m, causal):
        return "causal"
    return "generic"


def make_in_maps(q, k, v, mask, Wq, bq, Wk, bk, Wv, bv, Wo, bo, mask_mode,
                 s=S):
    c32 = lambda a: np.ascontiguousarray(a, dtype=np.float32)
    # one host-side transpose per (batch, tensor), shared by the TP group
    xT = [[c32(np.asarray(t)[g].T) for t in (q, k, v)] for g in range(DP)]
    in_maps = []
    for c in range(N_CORES):
        g, r = c // TP, c % TP
        sl = slice(r * DHH, (r + 1) * DHH)
        m = {
            "xq": xT[g][0], "xk": xT[g][1], "xv": xT[g][2],
            "wq": c32(Wq[:, sl]), "wk": c32(Wk[:, sl]), "wv": c32(Wv[:, sl]),
            "wo": c32(Wo[sl, :]),
            "bq": c32(bq[sl]).reshape(1, DHH),
            "bk": c32(bk[sl]).reshape(1, DHH),
            "bv": c32(bv[sl]).reshape(1, DHH),
        }
        if mask_mode == "generic":
            # pre-scaled by sqrt(DH) so exp((s + m*8)/8) == exp(s/8 + m)
            m["maskT"] = c32(
                np.asarray(mask).reshape(s, s).T * np.float32(DH) ** 0.5
            )
        in_maps.append(m)
    return in_maps


def assemble(results, bo, s=S):
    out = np.empty((B, s, D), np.float32)
    piece = FD // TP  # 128 rows per (chunk, core)
    for c in range(N_CORES):
        g, r = c // TP, c % TP
        shard = np.asarray(results[c]["out"]).reshape(-1, piece, D)
        for qc in range(s // FD):
            out[g, qc * FD + r * piece:qc * FD + (r + 1) * piece, :] = (
                shard[qc]
            )
    out += np.asarray(bo, dtype=np.float32)[None, None, :]
    return out


_cache = {}
MM_DTYPE = "f32r"   # default compute dtype for kernel()


def kernel(q, k, v, mask, Wq, bq, Wk, bk, Wv, bv, Wo, bo):
    mask_mode = detect_mask_mode(mask)
    if mask_mode not in _cache:
        _cache[mask_mode] = build(mask_mode=mask_mode, mm_dtype=MM_DTYPE)
    nc = _cache[mask_mode]
    in_maps = make_in_maps(
        q, k, v, mask, Wq, bq, Wk, bk, Wv, bv, Wo, bo, mask_mode
    )
    res = run_bass_kernel_spmd(nc, in_maps, list(range(N_CORES)))
    return assemble(res.results, bo)
